# revision 2
# baseline (speedup 1.0000x reference)
"""Trainium2 Bass kernel for nn_MultiHeadClassifier.

  logits[b, c] = sum_{(g,l): label_ids[g,l]==c} group_probs[b,g] *
                 (features[b] @ W[g,l] + b[g,l])

Data-parallel over batch (8 cores, 4096 rows each). Per core:
  * Host prep: transpose features/group_probs; sort the G*L=1024 head
    outputs by target class, pad so no class straddles a 128-row chunk
    -> NCH chunks with disjoint class bands covering [0, C).
  * GEMM1 (PE, bf16): glT[gl, b] = Wsorted^T.T @ X^T per chunk/b-tile.
  * M-matmul (PE, bf16): MT[gl, b] = E_j.T @ pT (group-prob gather as a
    0/1 matmul).
  * ACT: per-partition bias add + PSUM drain; DVE: weighted = gb * MT.
  * Scatter (PE, bf16): logits[b, lo_j:hi_j] = weightedT_j.T @ S_j with
    S_j a 0/1 band matrix; bands disjoint -> independent start=True
    matmuls, accumulation happens inside the band via duplicate class
    columns of S.
"""
import os
import sys
import numpy as np
import ml_dtypes

for _p in ("/opt/trn_rl_repo",):
    if _p not in sys.path:
        sys.path.append(_p)

import concourse.bass as bass  # noqa: E402
import concourse.tile as tile  # noqa: E402
from concourse import bacc, mybir, bass_utils  # noqa: E402
from contextlib import ExitStack  # noqa: E402

F32 = mybir.dt.float32
BF16 = mybir.dt.bfloat16

B, F, G, L, C = 32768, 512, 16, 64, 1000
NCORE = 8
BC = B // NCORE          # 4096 batch rows per core
NT = BC // 512           # 8 b-tiles of 512
KF = F // 128            # 4 feature chunks

LAST_EXEC_NS = None


def _host_prep(W, b, label_ids):
    lab = np.asarray(label_ids).reshape(-1).astype(np.int64)
    GL = lab.shape[0]
    order = np.argsort(lab, kind="stable")
    rows, cur = [], 0
    classes, starts = np.unique(lab[order], return_index=True)
    starts = list(starts) + [GL]
    for ci in range(len(classes)):
        seg = order[starts[ci]:starts[ci + 1]]
        if cur + len(seg) > 128:
            rows += [-1] * (128 - cur)
            cur = 0
        rows += list(seg)
        cur = (cur + len(seg)) % 128
    if len(rows) % 128:
        rows += [-1] * (128 - len(rows) % 128)
    rows = np.array(rows, dtype=np.int64)
    K_pad = len(rows)
    NCH = K_pad // 128

    his = []
    for j in range(NCH):
        rj = rows[j * 128:(j + 1) * 128]
        valid = rj[rj >= 0]
        his.append(int(lab[valid].max()) + 1 if len(valid) else (his[-1] if his else 0))
    his[-1] = C
    for j in range(1, NCH):
        his[j] = max(his[j], his[j - 1])
    los = [0] + his[:-1]
    bands = list(zip(los, his))

    S_cat = np.zeros((128, C), dtype=ml_dtypes.bfloat16)
    for j, (lo, hi) in enumerate(bands):
        rj = rows[j * 128:(j + 1) * 128]
        for r in range(128):
            gl = rj[r]
            if gl >= 0:
                S_cat[r, lab[gl]] = 1.0

    Wflat = np.asarray(W).reshape(GL, F)
    bflat = np.asarray(b).reshape(GL)
    WT = np.zeros((F, K_pad), dtype=np.float32)
    biasT = np.zeros((128, NCH), dtype=np.float32)
    E = np.zeros((16, K_pad), dtype=ml_dtypes.bfloat16)
    for p, gl in enumerate(rows):
        if gl >= 0:
            WT[:, p] = Wflat[gl]
            biasT[p % 128, p // 128] = bflat[gl]
            E[gl // L, p] = 1.0
    return dict(K_pad=K_pad, NCH=NCH, bands=bands, S_cat=S_cat,
                WT=WT.astype(ml_dtypes.bfloat16), biasT=biasT, E=E)


def _band_segments(lo, hi):
    """Split [lo, hi) at 512-column (PSUM bank) boundaries."""
    segs = []
    while lo < hi:
        nxt = min(hi, (lo // 512 + 1) * 512)
        segs.append((lo, nxt))
        lo = nxt
    return segs


def _build_program(NCH, bands):
    nc = bacc.Bacc("TRN2", target_bir_lowering=False, debug=False,
                   num_devices=NCORE)
    xt_d = nc.dram_tensor("xt", [F, BC], BF16, kind="ExternalInput").ap()
    pt_d = nc.dram_tensor("pt", [16, BC], BF16, kind="ExternalInput").ap()
    wt_d = nc.dram_tensor("wt", [F, NCH * 128], BF16, kind="ExternalInput").ap()
    e_d = nc.dram_tensor("e", [16, NCH * 128], BF16, kind="ExternalInput").ap()
    bt_d = nc.dram_tensor("bt", [128, NCH], F32, kind="ExternalInput").ap()
    s_d = nc.dram_tensor("s", [128, C], BF16, kind="ExternalInput").ap()
    out_d = nc.dram_tensor("logits", [BC, C], F32, kind="ExternalOutput").ap()

    with tile.TileContext(nc) as tc, ExitStack() as ctx:
        const = ctx.enter_context(tc.tile_pool(name="const", bufs=1))
        psG = ctx.enter_context(tc.tile_pool(name="psG", bufs=2, space="PSUM"))
        psM = ctx.enter_context(tc.tile_pool(name="psM", bufs=2, space="PSUM"))
        psL = ctx.enter_context(tc.tile_pool(name="psL", bufs=2, space="PSUM"))
        sbG = ctx.enter_context(tc.tile_pool(name="sbG", bufs=6))
        sbW = ctx.enter_context(tc.tile_pool(name="sbW", bufs=24))
        sbO = ctx.enter_context(tc.tile_pool(name="sbO", bufs=6))

        # X^T column-slices: separate tiles for precise DMA deps.
        xts = [[None] * NT for _ in range(KF)]

        def load_x(k, t):
            t_ = const.tile([128, 512], BF16, name=f"x{k}_{t}", tag=f"x{k}_{t}")
            nc.sync.dma_start(t_[:],
                              xt_d[k * 128:(k + 1) * 128, bass.ts(t, 512)])
            xts[k][t] = t_

        # interleave the tiles needed by the first GEMM (x slices of t=0 and
        # W chunks) so the PE can start as early as possible
        wts = []
        for k in range(KF):
            load_x(k, 0)
            t_ = const.tile([128, NCH * 128], BF16, name=f"wts{k}", tag=f"wts{k}")
            nc.gpsimd.dma_start(t_[:], wt_d[k * 128:(k + 1) * 128, :])
            wts.append(t_)
        pts = const.tile([16, BC], BF16, name="pts", tag="pts")
        nc.gpsimd.dma_start(pts[:], pt_d[:])
        es = const.tile([16, NCH * 128], BF16, name="es", tag="es")
        nc.gpsimd.dma_start(es[:], e_d[:])
        bts = const.tile([128, NCH], F32, name="bts", tag="bts")
        nc.gpsimd.dma_start(bts[:], bt_d[:])
        ss = const.tile([128, C], BF16, name="ss", tag="ss")
        nc.gpsimd.dma_start(ss[:], s_d[:])
        for t in range(1, NT):
            for k in range(KF):
                load_x(k, t)

        all_wtjs = {}

        def gemm_phase(t):
            bsl = bass.ts(t, 512)
            wtjs = []
            for j in range(NCH):
                jsl = bass.ts(j, 128)
                pg = psG.tile([128, 512], F32, name="pg", tag="pg")
                for k in range(KF):
                    nc.tensor.matmul(pg[:], wts[k][:, jsl], xts[k][t][:],
                                     start=(k == 0), stop=(k == KF - 1))
                pm = psM.tile([128, 512], F32, name="pm", tag="pm")
                nc.tensor.matmul(pm[:], es[:, jsl], pts[:, bsl],
                                 start=True, stop=True)
                gb = sbG.tile([128, 512], BF16, name="gb", tag="gb")
                nc.scalar.activation(gb[:], pg[:],
                                     mybir.ActivationFunctionType.Identity,
                                     bias=bts[:, j:j + 1], scale=1.0)
                wtj = sbW.tile([128, 512], BF16, name="wtj", tag="wtj")
                nc.vector.tensor_mul(wtj[:], gb[:], pm[:])
                wtjs.append(wtj)
            all_wtjs[t] = wtjs

        def scatter_phase(t):
            wtjs = all_wtjs.pop(t)
            for bs_i in range(4):
                pl = psL.tile([128, 1024], F32, name="pl", tag="pl")
                for j, (lo, hi) in enumerate(bands):
                    for (n0, n1) in _band_segments(lo, hi):
                        nc.tensor.matmul(pl[:, n0:n1],
                                         wtjs[j][:, bass.ts(bs_i, 128)],
                                         ss[:, n0:n1], start=True, stop=True)
                ob = sbO.tile([128, C], F32, name="ob", tag="ob")
                # split the PSUM drain per bank across both engines
                nc.scalar.activation(ob[:, :512], pl[:, :512],
                                     mybir.ActivationFunctionType.Identity,
                                     bias=0.0, scale=1.0)
                nc.vector.tensor_copy(ob[:, 512:C], pl[:, 512:C])
                # scalar-queue HWDGE: keep output stream off the input queue
                nc.scalar.dma_start(out_d[t * 512 + bs_i * 128:
                                          t * 512 + (bs_i + 1) * 128, :], ob[:])

        # software-pipelined emission: scatter(t-1) after gemm(t)
        for t in range(NT + 1):
            if t < NT:
                gemm_phase(t)
            if t > 0:
                scatter_phase(t - 1)
    nc.finalize()
    return nc


def kernel(features, group_probs, W, b, label_ids):
    global LAST_EXEC_NS
    features = np.asarray(features, dtype=np.float32)
    group_probs = np.asarray(group_probs, dtype=np.float32)
    prep = _host_prep(W, b, label_ids)
    nc = _build_program(prep["NCH"], prep["bands"])

    XT = np.ascontiguousarray(features.T.astype(ml_dtypes.bfloat16))
    PT = np.ascontiguousarray(group_probs.T.astype(ml_dtypes.bfloat16))
    in_maps = []
    for c in range(NCORE):
        in_maps.append({
            "xt": np.ascontiguousarray(XT[:, c * BC:(c + 1) * BC]),
            "pt": np.ascontiguousarray(PT[:, c * BC:(c + 1) * BC]),
            "wt": prep["WT"],
            "e": prep["E"],
            "bt": prep["biasT"],
            "s": prep["S_cat"],
        })

    trace = bool(os.environ.get("BASS_TRACE"))
    if trace:
        bass_utils.upload_artifacts = lambda d: "local://skipped"
    try:
        res = bass_utils.run_bass_kernel_spmd(nc, in_maps,
                                              core_ids=list(range(NCORE)))
    except Exception:
        # transient NRT device errors have been observed; one retry
        res = bass_utils.run_bass_kernel_spmd(nc, in_maps,
                                              core_ids=list(range(NCORE)))
    if trace:
        LAST_EXEC_NS = res.exec_time_ns
        if res.exec_time_ns is not None:
            print(f"HW exec time: {res.exec_time_ns} ns")
        if res.instructions_and_trace is not None:
            print(f"Trace path: {res.instructions_and_trace[1]}")
        if res.profile_json is not None:
            print(f"Profile json: {res.profile_json}")

    out = np.concatenate([res.results[c]["logits"] for c in range(NCORE)],
                         axis=0)
    return np.ascontiguousarray(out.astype(np.float32))



# revision 4
# speedup vs baseline: 1.1313x; 1.1313x over previous
"""Trainium2 Bass kernel for nn_MultiHeadClassifier.

  logits[b, c] = sum_{(g,l): label_ids[g,l]==c} group_probs[b,g] *
                 (features[b] @ W[g,l] + b[g,l])

Data-parallel over batch (8 cores, 4096 rows each). Per core:
  * Host prep: transpose features/group_probs; sort the G*L=1024 head
    outputs by target class with NO padding (K_pad=1024, 8 chunks).
    Chunk class-bands may overlap by one boundary class; the scatter
    relies on per-element PSUM has_written semantics (accumulate where
    written, overwrite where pending) so overlapping bands need no
    extra matmuls.  Even/odd chunks use separate S matrices so a
    boundary class column never mixes rows of adjacent chunks.
  * Warmup: 12 junk matmuls at t=0 to lift the PE HAM clock gate
    (1.2 -> 2.4 GHz) before real work arrives.
  * GEMM (PE, bf16): per (t-pair, chunk j) 8 matmuls accumulate
    glT[128, 1024] over 4 K-chunks into a 2-bank PSUM pair tile.
  * M (PE): MT[gl, b] = E_j.T @ pT as a 0/1 matmul into a paired
    [128, 1024] PSUM tile.
  * ACT: one [128,1024] activation per (tp, j) drains PSUM with the
    per-partition bias; DVE: weighted = gb * MT (one [128,1024] mul).
  * Scatter (PE, bf16): per 128-row b-subtile and 512-col class half,
    band matmuls of wtj.T @ S accumulate into a single-bank psL tile.
  * Drains alternate ACT/DVE; output staged bf16 in [128, 4000] tiles
    and DMAd (8KB/partition descriptors) alternating the two HWDGE
    queues (sync/scalar) to a [128, 32000] DRAM layout that the host
    unpermutes.
"""
import os
import sys
import numpy as np
import ml_dtypes

for _p in ("/opt/trn_rl_repo",):
    if _p not in sys.path:
        sys.path.append(_p)

import concourse.bass as bass  # noqa: E402
import concourse.tile as tile  # noqa: E402
from concourse import bacc, mybir, bass_utils  # noqa: E402
from contextlib import ExitStack  # noqa: E402

F32 = mybir.dt.float32
BF16 = mybir.dt.bfloat16

B, F, G, L, C = 32768, 512, 16, 64, 1000
NCORE = 8
BC = B // NCORE          # 4096 batch rows per core
NT = BC // 512           # 8 b-tiles of 512
NP = NT // 2             # 4 t-pairs of 1024
KF = F // 128            # 4 feature chunks
NCH = (G * L) // 128     # 8 sorted-head chunks, no padding

LAST_EXEC_NS = None


def _host_prep(W, b, label_ids):
    lab = np.asarray(label_ids).reshape(-1).astype(np.int64)
    order = np.argsort(lab, kind="stable")
    slab = lab[order]

    bands = []
    lo = 0
    for j in range(NCH):
        last_c = int(slab[j * 128 + 127])
        hi = C if j == NCH - 1 else last_c + 1
        assert hi > lo
        bands.append((lo, hi))
        if j < NCH - 1:
            lo = last_c if int(slab[(j + 1) * 128]) == last_c else last_c + 1

    S = np.zeros((2, 128, C), dtype=ml_dtypes.bfloat16)
    for j in range(NCH):
        for r in range(128):
            S[j % 2, r, int(slab[j * 128 + r])] = 1.0

    Wflat = np.asarray(W).reshape(G * L, F)[order]
    WT = np.ascontiguousarray(Wflat.T.astype(ml_dtypes.bfloat16))
    bias = np.asarray(b).reshape(G * L)[order]
    biasT = np.zeros((128, NCH), dtype=np.float32)
    E = np.zeros((16, G * L), dtype=ml_dtypes.bfloat16)
    for p in range(G * L):
        biasT[p % 128, p // 128] = bias[p]
        E[order[p] // L, p] = 1.0
    return dict(bands=bands, S0=np.ascontiguousarray(S[0]),
                S1=np.ascontiguousarray(S[1]), WT=WT, biasT=biasT, E=E)


def _scatter_plans(bands):
    """Per class-half (bank) the ordered segment list (j, n0, n1, start, stop)."""
    halves = [[], []]
    for j, (lo, hi) in enumerate(bands):
        x = lo
        while x < hi:
            nxt = min(hi, (x // 512 + 1) * 512)
            halves[x // 512].append((j, x, nxt))
            x = nxt
    plans = []
    for h in (0, 1):
        segs = halves[h]
        plans.append([(j, n0, n1, i == 0, i == len(segs) - 1)
                      for i, (j, n0, n1) in enumerate(segs)])
    return plans


def _build_program(bands):
    plans = _scatter_plans(bands)
    nc = bacc.Bacc("TRN2", target_bir_lowering=False, debug=False,
                   num_devices=NCORE)
    xt_d = nc.dram_tensor("xt", [F, BC], BF16, kind="ExternalInput").ap()
    pt_d = nc.dram_tensor("pt", [16, BC], BF16, kind="ExternalInput").ap()
    wt_d = nc.dram_tensor("wt", [F, NCH * 128], BF16, kind="ExternalInput").ap()
    e_d = nc.dram_tensor("e", [16, NCH * 128], BF16, kind="ExternalInput").ap()
    bt_d = nc.dram_tensor("bt", [128, NCH], F32, kind="ExternalInput").ap()
    s0_d = nc.dram_tensor("s0", [128, C], BF16, kind="ExternalInput").ap()
    s1_d = nc.dram_tensor("s1", [128, C], BF16, kind="ExternalInput").ap()
    out_d = nc.dram_tensor("logits", [128, NP * 2 * 4000], BF16,
                           kind="ExternalOutput").ap()

    with tile.TileContext(nc) as tc, ExitStack() as ctx:
        const = ctx.enter_context(tc.tile_pool(name="const", bufs=1))
        psG = ctx.enter_context(tc.tile_pool(name="psG", bufs=2, space="PSUM"))
        psM = ctx.enter_context(tc.tile_pool(name="psM", bufs=1, space="PSUM"))
        psL = ctx.enter_context(tc.tile_pool(name="psL", bufs=2, space="PSUM"))
        sbG = ctx.enter_context(tc.tile_pool(name="sbG", bufs=4))
        sbW = ctx.enter_context(tc.tile_pool(name="sbW", bufs=18))
        sbO = ctx.enter_context(tc.tile_pool(name="sbO", bufs=3))

        junk = const.tile([128, 512], BF16, name="junk", tag="junk")
        nc.gpsimd.memset(junk[:], 0.0)

        # input loads: xt on sync, weights/probs on gpsimd, S on scalar
        xta, xtb, wts = [], [], []
        for k in range(KF):
            t_ = const.tile([128, 1024], BF16, name=f"xta{k}", tag=f"xta{k}")
            nc.sync.dma_start(t_[:], xt_d[k * 128:(k + 1) * 128, 0:1024])
            xta.append(t_)
        for k in range(KF):
            t_ = const.tile([128, NCH * 128], BF16, name=f"wts{k}",
                            tag=f"wts{k}")
            nc.gpsimd.dma_start(t_[:], wt_d[k * 128:(k + 1) * 128, :])
            wts.append(t_)
        es = const.tile([16, NCH * 128], BF16, name="es", tag="es")
        nc.gpsimd.dma_start(es[:], e_d[:])
        bts = const.tile([128, NCH], F32, name="bts", tag="bts")
        nc.gpsimd.dma_start(bts[:], bt_d[:])
        pts = const.tile([16, BC], BF16, name="pts", tag="pts")
        nc.gpsimd.dma_start(pts[:], pt_d[:])
        ss = []
        for i, sd in enumerate((s0_d, s1_d)):
            t_ = const.tile([128, C], BF16, name=f"ss{i}", tag=f"ss{i}")
            nc.scalar.dma_start(t_[:], sd[:])
            ss.append(t_)
        for k in range(KF):
            t_ = const.tile([128, 3072], BF16, name=f"xtb{k}", tag=f"xtb{k}")
            nc.sync.dma_start(t_[:], xt_d[k * 128:(k + 1) * 128, 1024:BC])
            xtb.append(t_)

        # HAM warmup: junk matmuls into a psG-pool tile (same tag -> no
        # extra PSUM footprint; the bank is recycled by the real GEMMs)
        warm = psG.tile([128, 1024], F32, name="psg", tag="psg")
        for _ in range(12):
            nc.tensor.matmul(warm[:, 0:512], junk[:, 0:128], junk[:],
                             start=True, stop=True)

        def x_slice(k, tp, h):
            off = tp * 1024 + h * 512
            if off < 1024:
                return xta[k][:, off:off + 512]
            return xtb[k][:, off - 1024:off - 1024 + 512]

        wtjs = {}
        obs = {}
        drain_ct = [0]

        def scatter_unit(tpp, u):
            bs, h = u // 2, u % 2
            hb = bs // 4
            if u % 8 == 0:
                obs[(tpp, hb)] = sbO.tile([128, 4000], BF16, name="ob",
                                          tag="ob")
            ob = obs[(tpp, hb)]
            w = 512 if h == 0 else C - 512
            pl = psL.tile([128, 512], F32, name="pl", tag="pl")
            for (j, n0, n1, st, sp) in plans[h]:
                nc.tensor.matmul(pl[:, n0 - h * 512:n1 - h * 512],
                                 wtjs[(tpp, j)][:, bs * 128:(bs + 1) * 128],
                                 ss[j % 2][:, n0:n1], start=st, stop=sp,
                                 skip_group_check=True)
            dst = ob[:, (bs % 4) * 1000 + h * 512:
                     (bs % 4) * 1000 + h * 512 + w]
            if drain_ct[0] % 2 == 0:
                nc.scalar.activation(dst, pl[:, 0:w],
                                     mybir.ActivationFunctionType.Identity,
                                     bias=0.0, scale=1.0)
            else:
                nc.vector.tensor_copy(dst, pl[:, 0:w])
            drain_ct[0] += 1
            if u % 8 == 7:
                d = tpp * 2 + hb
                eng = nc.sync if d % 2 == 0 else nc.scalar
                eng.dma_start(out_d[:, d * 4000:(d + 1) * 4000], ob[:])
                del obs[(tpp, hb)]

        for tp in range(NP):
            for j in range(NCH):
                jsl = bass.ts(j, 128)
                psg = psG.tile([128, 1024], F32, name="psg", tag="psg")
                for k in range(KF):
                    for h in (0, 1):
                        nc.tensor.matmul(psg[:, h * 512:(h + 1) * 512],
                                         wts[k][:, jsl], x_slice(k, tp, h),
                                         start=(k == 0), stop=(k == KF - 1))
                pm = psM.tile([128, 1024], F32, name="pm", tag="pm")
                for h in (0, 1):
                    nc.tensor.matmul(pm[:, h * 512:(h + 1) * 512],
                                     es[:, jsl],
                                     pts[:, bass.ts(tp * 2 + h, 512)],
                                     start=True, stop=True)
                gb = sbG.tile([128, 1024], BF16, name="gb", tag="gb")
                nc.scalar.activation(gb[:], psg[:],
                                     mybir.ActivationFunctionType.Identity,
                                     bias=bts[:, j:j + 1], scale=1.0)
                wtj = sbW.tile([128, 1024], BF16, name="wtj", tag="wtj")
                nc.vector.tensor_mul(wtj[:], gb[:], pm[:])
                wtjs[(tp, j)] = wtj
                if tp > 0:
                    scatter_unit(tp - 1, 2 * j)
                    scatter_unit(tp - 1, 2 * j + 1)
            if tp > 0:
                for jj in range(NCH):
                    del wtjs[(tp - 1, jj)]
        for u in range(16):
            scatter_unit(NP - 1, u)
    nc.finalize()
    return nc


def kernel(features, group_probs, W, b, label_ids):
    global LAST_EXEC_NS
    features = np.asarray(features, dtype=np.float32)
    group_probs = np.asarray(group_probs, dtype=np.float32)
    prep = _host_prep(W, b, label_ids)
    nc = _build_program(prep["bands"])

    XT = np.ascontiguousarray(features.T.astype(ml_dtypes.bfloat16))
    PT = np.ascontiguousarray(group_probs.T.astype(ml_dtypes.bfloat16))
    in_maps = []
    for c in range(NCORE):
        in_maps.append({
            "xt": np.ascontiguousarray(XT[:, c * BC:(c + 1) * BC]),
            "pt": np.ascontiguousarray(PT[:, c * BC:(c + 1) * BC]),
            "wt": prep["WT"],
            "e": prep["E"],
            "bt": prep["biasT"],
            "s0": prep["S0"],
            "s1": prep["S1"],
        })

    trace = bool(os.environ.get("BASS_TRACE"))
    if trace:
        bass_utils.upload_artifacts = lambda d: "local://skipped"
    try:
        res = bass_utils.run_bass_kernel_spmd(nc, in_maps,
                                              core_ids=list(range(NCORE)))
    except Exception:
        # transient NRT device errors have been observed; one retry
        res = bass_utils.run_bass_kernel_spmd(nc, in_maps,
                                              core_ids=list(range(NCORE)))
    if trace:
        LAST_EXEC_NS = res.exec_time_ns
        if res.exec_time_ns is not None:
            print(f"HW exec time: {res.exec_time_ns} ns")
        if res.instructions_and_trace is not None:
            print(f"Trace path: {res.instructions_and_trace[1]}")
        if res.profile_json is not None:
            print(f"Profile json: {res.profile_json}")

    parts = []
    for c in range(NCORE):
        arr = np.asarray(res.results[c]["logits"]).astype(np.float32)
        arr = arr.reshape(128, NP, 2, 4, 1000)
        parts.append(arr.transpose(1, 2, 3, 0, 4).reshape(BC, C))
    return np.ascontiguousarray(np.concatenate(parts, axis=0))


# revision 13
# speedup vs baseline: 1.1345x; 1.0028x over previous
"""Trainium2 Bass kernel for nn_MultiHeadClassifier.

  logits[b, c] = sum_{(g,l): label_ids[g,l]==c} group_probs[b,g] *
                 (features[b] @ W[g,l] + b[g,l])

Data-parallel over batch (8 cores, 4096 rows each). Per core:
  * Host prep: transpose features/group_probs; sort the G*L=1024 head
    outputs by target class with NO padding (K_pad=1024, 8 chunks).
    Chunk class-bands may overlap by one boundary class; the scatter
    relies on per-element PSUM has_written semantics (accumulate where
    written, overwrite where pending) so overlapping bands need no
    extra matmuls.  Even/odd chunks use separate S matrices so a
    boundary class column never mixes rows of adjacent chunks.
  * Warmup: 12 junk matmuls at t=0 to lift the PE HAM clock gate
    (1.2 -> 2.4 GHz) before real work arrives.
  * GEMM (PE, bf16): per (t-pair, chunk j) 8 matmuls accumulate
    glT[128, 1024] over 4 K-chunks into a 2-bank PSUM pair tile.
  * M (PE): MT[gl, b] = E_j.T @ pT as a 0/1 matmul into a paired
    [128, 1024] PSUM tile.
  * ACT: one [128,1024] activation per (tp, j) drains PSUM with the
    per-partition bias; DVE: weighted = gb * MT (one [128,1024] mul).
  * Scatter (PE, bf16): per 128-row b-subtile and 512-col class half,
    band matmuls of wtj.T @ S accumulate into a single-bank psL tile.
  * Drains alternate ACT/DVE; output staged bf16 in [128, 4000] tiles
    and DMAd (8KB/partition descriptors) alternating the two HWDGE
    queues (sync/scalar) to a [128, 32000] DRAM layout that the host
    unpermutes.
"""
import os
import sys
import numpy as np
import ml_dtypes

for _p in ("/opt/trn_rl_repo",):
    if _p not in sys.path:
        sys.path.append(_p)

import concourse.bass as bass  # noqa: E402
import concourse.tile as tile  # noqa: E402
from concourse import bacc, mybir, bass_utils  # noqa: E402
from contextlib import ExitStack  # noqa: E402

F32 = mybir.dt.float32
BF16 = mybir.dt.bfloat16

B, F, G, L, C = 32768, 512, 16, 64, 1000
NCORE = 8
BC = B // NCORE          # 4096 batch rows per core
NT = BC // 512           # 8 b-tiles of 512
NP = NT // 2             # 4 t-pairs of 1024
KF = F // 128            # 4 feature chunks
NCH = (G * L) // 128     # 8 sorted-head chunks, no padding

LAST_EXEC_NS = None


def _host_prep(W, b, label_ids):
    lab = np.asarray(label_ids).reshape(-1).astype(np.int64)
    order = np.argsort(lab, kind="stable")
    slab = lab[order]

    bands = []
    lo = 0
    for j in range(NCH):
        last_c = int(slab[j * 128 + 127])
        hi = C if j == NCH - 1 else last_c + 1
        assert hi > lo
        bands.append((lo, hi))
        if j < NCH - 1:
            lo = last_c if int(slab[(j + 1) * 128]) == last_c else last_c + 1

    S = np.zeros((2, 128, C), dtype=ml_dtypes.bfloat16)
    for j in range(NCH):
        for r in range(128):
            S[j % 2, r, int(slab[j * 128 + r])] = 1.0

    Wflat = np.asarray(W).reshape(G * L, F)[order]
    WT = np.ascontiguousarray(Wflat.T.astype(ml_dtypes.bfloat16))
    bias = np.asarray(b).reshape(G * L)[order]
    biasT = np.zeros((128, NCH), dtype=np.float32)
    # dma_gather index planes: idx i lives at [i % 16, i // 16]; rows
    # 16..127 are ignored by HW but must hold valid (>= -1, < 16) values
    E = np.zeros((16, G * L), dtype=ml_dtypes.bfloat16)
    for p in range(G * L):
        biasT[p % 128, p // 128] = bias[p]
        E[order[p] // L, p] = 1.0
    return dict(bands=bands, S0=np.ascontiguousarray(S[0]),
                S1=np.ascontiguousarray(S[1]), WT=WT, biasT=biasT, E=E)


def _scatter_plans(bands):
    """Per class-half (bank) the ordered segment list (j, n0, n1, start, stop)."""
    halves = [[], []]
    for j, (lo, hi) in enumerate(bands):
        x = lo
        while x < hi:
            nxt = min(hi, (x // 512 + 1) * 512)
            halves[x // 512].append((j, x, nxt))
            x = nxt
    plans = []
    for h in (0, 1):
        segs = halves[h]
        plans.append([(j, n0, n1, i == 0, i == len(segs) - 1)
                      for i, (j, n0, n1) in enumerate(segs)])
    return plans


def _build_program(bands):
    plans = _scatter_plans(bands)
    nc = bacc.Bacc("TRN2", target_bir_lowering=False, debug=False,
                   num_devices=NCORE)
    xt_d = nc.dram_tensor("xt", [F, BC], BF16, kind="ExternalInput").ap()
    pt_d = nc.dram_tensor("pt", [16, BC], BF16, kind="ExternalInput").ap()
    wt_d = nc.dram_tensor("wt", [F, NCH * 128], BF16, kind="ExternalInput").ap()
    e_d = nc.dram_tensor("e", [16, NCH * 128], BF16, kind="ExternalInput").ap()
    bt_d = nc.dram_tensor("bt", [128, NCH], F32, kind="ExternalInput").ap()
    s0_d = nc.dram_tensor("s0", [128, C], BF16, kind="ExternalInput").ap()
    s1_d = nc.dram_tensor("s1", [128, C], BF16, kind="ExternalInput").ap()
    out_d = nc.dram_tensor("logits", [128, NP * 2 * 4000], BF16,
                           kind="ExternalOutput").ap()

    with tile.TileContext(nc) as tc, ExitStack() as ctx:
        const = ctx.enter_context(tc.tile_pool(name="const", bufs=1))
        psG = ctx.enter_context(tc.tile_pool(name="psG", bufs=2, space="PSUM"))
        psM = ctx.enter_context(tc.tile_pool(name="psM", bufs=1, space="PSUM"))
        psL = ctx.enter_context(tc.tile_pool(name="psL", bufs=2, space="PSUM"))
        sbG = ctx.enter_context(tc.tile_pool(name="sbG", bufs=4))
        sbW = ctx.enter_context(tc.tile_pool(name="sbW", bufs=18))
        sbO = ctx.enter_context(tc.tile_pool(name="sbO", bufs=3))

        junk = const.tile([128, 512], BF16, name="junk", tag="junk")
        nc.vector.memset(junk[:], 0.0)

        es = const.tile([16, NCH * 128], BF16, name="es", tag="es")
        nc.gpsimd.dma_start(es[:], e_d[:])
        pts = const.tile([16, BC], BF16, name="pts", tag="pts")
        nc.gpsimd.dma_start(pts[:], pt_d[:])
        # xt on sync; weights/bias/S on scalar
        xta, xtb, wts = [], [], []
        for k in range(KF):
            t_ = const.tile([128, 1024], BF16, name=f"xta{k}", tag=f"xta{k}")
            nc.sync.dma_start(t_[:], xt_d[k * 128:(k + 1) * 128, 0:1024])
            xta.append(t_)
        for k in range(KF):
            t_ = const.tile([128, NCH * 128], BF16, name=f"wts{k}",
                            tag=f"wts{k}")
            nc.scalar.dma_start(t_[:], wt_d[k * 128:(k + 1) * 128, :])
            wts.append(t_)
        bts = const.tile([128, NCH], F32, name="bts", tag="bts")
        nc.scalar.dma_start(bts[:], bt_d[:])
        ss = []
        for i, sd in enumerate((s0_d, s1_d)):
            t_ = const.tile([128, C], BF16, name=f"ss{i}", tag=f"ss{i}")
            nc.scalar.dma_start(t_[:], sd[:])
            ss.append(t_)
        for k in range(KF):
            t_ = const.tile([128, 3072], BF16, name=f"xtb{k}", tag=f"xtb{k}")
            nc.sync.dma_start(t_[:], xt_d[k * 128:(k + 1) * 128, 1024:BC])
            xtb.append(t_)
        # HAM warmup: junk matmuls into a psG-pool tile (same tag -> no
        # extra PSUM footprint; the bank is recycled by the real GEMMs)
        warm = psG.tile([128, 1024], F32, name="psg", tag="psg")
        for _ in range(12):
            nc.tensor.matmul(warm[:, 0:512], junk[:, 0:128], junk[:],
                             start=True, stop=True)

        def x_slice(k, tp, h):
            off = tp * 1024 + h * 512
            if off < 1024:
                return xta[k][:, off:off + 512]
            return xtb[k][:, off - 1024:off - 1024 + 512]

        wtjs = {}
        obs = {}
        drain_ct = [0]

        def scatter_unit(tpp, u):
            bs, h = u // 2, u % 2
            hb = bs // 4
            if u % 8 == 0:
                obs[(tpp, hb)] = sbO.tile([128, 4000], BF16, name="ob",
                                          tag="ob")
            ob = obs[(tpp, hb)]
            w = 512 if h == 0 else C - 512
            pl = psL.tile([128, 512], F32, name="pl", tag="pl")
            for (j, n0, n1, st, sp) in plans[h]:
                nc.tensor.matmul(pl[:, n0 - h * 512:n1 - h * 512],
                                 wtjs[(tpp, j)][:, bs * 128:(bs + 1) * 128],
                                 ss[j % 2][:, n0:n1], start=st, stop=sp,
                                 skip_group_check=True)
            dst = ob[:, (bs % 4) * 1000 + h * 512:
                     (bs % 4) * 1000 + h * 512 + w]
            if drain_ct[0] % 2 == 0:
                nc.scalar.activation(dst, pl[:, 0:w],
                                     mybir.ActivationFunctionType.Identity,
                                     bias=0.0, scale=1.0)
            else:
                nc.vector.tensor_copy(dst, pl[:, 0:w])
            drain_ct[0] += 1
            if u % 8 == 7:
                d = tpp * 2 + hb
                eng = nc.sync if d % 2 == 0 else nc.scalar
                eng.dma_start(out_d[:, d * 4000:(d + 1) * 4000], ob[:])
                del obs[(tpp, hb)]

        for tp in range(NP):
            for j in range(NCH):
                jsl = bass.ts(j, 128)
                psg = psG.tile([128, 1024], F32, name="psg", tag="psg")
                for k in range(KF):
                    for h in (0, 1):
                        nc.tensor.matmul(psg[:, h * 512:(h + 1) * 512],
                                         wts[k][:, jsl], x_slice(k, tp, h),
                                         start=(k == 0), stop=(k == KF - 1))
                pm = psM.tile([128, 1024], F32, name="pm", tag="pm")
                for h in (0, 1):
                    nc.tensor.matmul(pm[:, h * 512:(h + 1) * 512],
                                     es[:, jsl],
                                     pts[:, bass.ts(tp * 2 + h, 512)],
                                     start=True, stop=True)
                gb = sbG.tile([128, 1024], BF16, name="gb", tag="gb")
                nc.scalar.activation(gb[:], psg[:],
                                     mybir.ActivationFunctionType.Identity,
                                     bias=bts[:, j:j + 1], scale=1.0)
                wtj = sbW.tile([128, 1024], BF16, name="wtj", tag="wtj")
                nc.vector.tensor_mul(wtj[:], gb[:], pm[:])
                wtjs[(tp, j)] = wtj
                if tp > 0:
                    scatter_unit(tp - 1, 2 * j)
                    scatter_unit(tp - 1, 2 * j + 1)
            if tp > 0:
                for jj in range(NCH):
                    del wtjs[(tp - 1, jj)]
        for u in range(16):
            scatter_unit(NP - 1, u)
    nc.finalize()
    return nc


def kernel(features, group_probs, W, b, label_ids):
    global LAST_EXEC_NS
    features = np.asarray(features, dtype=np.float32)
    group_probs = np.asarray(group_probs, dtype=np.float32)
    prep = _host_prep(W, b, label_ids)
    nc = _build_program(prep["bands"])

    XT = np.ascontiguousarray(features.T.astype(ml_dtypes.bfloat16))
    PT = np.ascontiguousarray(group_probs.T.astype(ml_dtypes.bfloat16))
    in_maps = []
    for c in range(NCORE):
        in_maps.append({
            "xt": np.ascontiguousarray(XT[:, c * BC:(c + 1) * BC]),
            "pt": np.ascontiguousarray(PT[:, c * BC:(c + 1) * BC]),
            "wt": prep["WT"],
            "e": prep["E"],
            "bt": prep["biasT"],
            "s0": prep["S0"],
            "s1": prep["S1"],
        })

    trace = bool(os.environ.get("BASS_TRACE"))
    if trace:
        bass_utils.upload_artifacts = lambda d: "local://skipped"
    try:
        res = bass_utils.run_bass_kernel_spmd(nc, in_maps,
                                              core_ids=list(range(NCORE)))
    except Exception:
        # transient NRT device errors have been observed; one retry
        res = bass_utils.run_bass_kernel_spmd(nc, in_maps,
                                              core_ids=list(range(NCORE)))
    if trace:
        LAST_EXEC_NS = res.exec_time_ns
        if res.exec_time_ns is not None:
            print(f"HW exec time: {res.exec_time_ns} ns")
        if res.instructions_and_trace is not None:
            print(f"Trace path: {res.instructions_and_trace[1]}")
        if res.profile_json is not None:
            print(f"Profile json: {res.profile_json}")

    parts = []
    for c in range(NCORE):
        arr = np.asarray(res.results[c]["logits"]).astype(np.float32)
        arr = arr.reshape(128, NP, 2, 4, 1000)
        parts.append(arr.transpose(1, 2, 3, 0, 4).reshape(BC, C))
    return np.ascontiguousarray(np.concatenate(parts, axis=0))


# revision 14
# speedup vs baseline: 1.1356x; 1.0009x over previous
"""Trainium2 Bass kernel for nn_MultiHeadClassifier.

  logits[b, c] = sum_{(g,l): label_ids[g,l]==c} group_probs[b,g] *
                 (features[b] @ W[g,l] + b[g,l])

Data-parallel over batch (8 cores, 4096 rows each). Per core:
  * Host prep: transpose features/group_probs; sort the G*L=1024 head
    outputs by target class with NO padding (K_pad=1024, 8 chunks).
    Chunk class-bands may overlap by one boundary class; the scatter
    relies on per-element PSUM has_written semantics (accumulate where
    written, overwrite where pending) so overlapping bands need no
    extra matmuls.  Even/odd chunks use separate S matrices so a
    boundary class column never mixes rows of adjacent chunks.
  * Warmup: 12 junk matmuls at t=0 to lift the PE HAM clock gate
    (1.2 -> 2.4 GHz) before real work arrives.
  * GEMM (PE, bf16): per (t-pair, chunk j) 8 matmuls accumulate
    glT[128, 1024] over 4 K-chunks into a 2-bank PSUM pair tile.
  * M (PE): MT[gl, b] = E_j.T @ pT as a 0/1 matmul into a paired
    [128, 1024] PSUM tile.
  * ACT: one [128,1024] activation per (tp, j) drains PSUM with the
    per-partition bias; DVE: weighted = gb * MT (one [128,1024] mul).
  * Scatter (PE, bf16): per 128-row b-subtile and 512-col class half,
    band matmuls of wtj.T @ S accumulate into a single-bank psL tile.
  * Drains alternate ACT/DVE; output staged bf16 in [128, 4000] tiles
    and DMAd (8KB/partition descriptors) alternating the two HWDGE
    queues (sync/scalar) to a [128, 32000] DRAM layout that the host
    unpermutes.
"""
import os
import sys
import numpy as np
import ml_dtypes

for _p in ("/opt/trn_rl_repo",):
    if _p not in sys.path:
        sys.path.append(_p)

import concourse.bass as bass  # noqa: E402
import concourse.tile as tile  # noqa: E402
from concourse import bacc, mybir, bass_utils  # noqa: E402
from contextlib import ExitStack  # noqa: E402

F32 = mybir.dt.float32
BF16 = mybir.dt.bfloat16

B, F, G, L, C = 32768, 512, 16, 64, 1000
NCORE = 8
BC = B // NCORE          # 4096 batch rows per core
NT = BC // 512           # 8 b-tiles of 512
NP = NT // 2             # 4 t-pairs of 1024
KF = F // 128            # 4 feature chunks
NCH = (G * L) // 128     # 8 sorted-head chunks, no padding

LAST_EXEC_NS = None


def _host_prep(W, b, label_ids):
    lab = np.asarray(label_ids).reshape(-1).astype(np.int64)
    order = np.argsort(lab, kind="stable")
    slab = lab[order]

    bands = []
    lo = 0
    for j in range(NCH):
        last_c = int(slab[j * 128 + 127])
        hi = C if j == NCH - 1 else last_c + 1
        assert hi > lo
        bands.append((lo, hi))
        if j < NCH - 1:
            lo = last_c if int(slab[(j + 1) * 128]) == last_c else last_c + 1

    S = np.zeros((2, 128, C), dtype=ml_dtypes.bfloat16)
    for j in range(NCH):
        for r in range(128):
            S[j % 2, r, int(slab[j * 128 + r])] = 1.0

    Wflat = np.asarray(W).reshape(G * L, F)[order]
    WT = np.ascontiguousarray(Wflat.T.astype(ml_dtypes.bfloat16))
    bias = np.asarray(b).reshape(G * L)[order]
    biasT = np.zeros((128, NCH), dtype=np.float32)
    # dma_gather index planes: idx i lives at [i % 16, i // 16]; rows
    # 16..127 are ignored by HW but must hold valid (>= -1, < 16) values
    E = np.zeros((16, G * L), dtype=ml_dtypes.bfloat16)
    for p in range(G * L):
        biasT[p % 128, p // 128] = bias[p]
        E[order[p] // L, p] = 1.0
    return dict(bands=bands, S0=np.ascontiguousarray(S[0]),
                S1=np.ascontiguousarray(S[1]), WT=WT, biasT=biasT, E=E)


def _scatter_plans(bands):
    """Per class-half (bank) the ordered segment list (j, n0, n1, start, stop)."""
    halves = [[], []]
    for j, (lo, hi) in enumerate(bands):
        x = lo
        while x < hi:
            nxt = min(hi, (x // 512 + 1) * 512)
            halves[x // 512].append((j, x, nxt))
            x = nxt
    plans = []
    for h in (0, 1):
        segs = halves[h]
        plans.append([(j, n0, n1, i == 0, i == len(segs) - 1)
                      for i, (j, n0, n1) in enumerate(segs)])
    return plans


def _build_program(bands):
    plans = _scatter_plans(bands)
    nc = bacc.Bacc("TRN2", target_bir_lowering=False, debug=False,
                   num_devices=NCORE)
    xt_d = nc.dram_tensor("xt", [F, BC], BF16, kind="ExternalInput").ap()
    pt_d = nc.dram_tensor("pt", [16, BC], BF16, kind="ExternalInput").ap()
    wt_d = nc.dram_tensor("wt", [F, NCH * 128], BF16, kind="ExternalInput").ap()
    e_d = nc.dram_tensor("e", [16, NCH * 128], BF16, kind="ExternalInput").ap()
    bt_d = nc.dram_tensor("bt", [128, NCH], F32, kind="ExternalInput").ap()
    s0_d = nc.dram_tensor("s0", [128, C], BF16, kind="ExternalInput").ap()
    s1_d = nc.dram_tensor("s1", [128, C], BF16, kind="ExternalInput").ap()
    out_d = nc.dram_tensor("logits", [128, NP * 2 * 4000], BF16,
                           kind="ExternalOutput").ap()

    with tile.TileContext(nc) as tc, ExitStack() as ctx:
        const = ctx.enter_context(tc.tile_pool(name="const", bufs=1))
        psG = ctx.enter_context(tc.tile_pool(name="psG", bufs=2, space="PSUM"))
        psM = ctx.enter_context(tc.tile_pool(name="psM", bufs=1, space="PSUM"))
        psL = ctx.enter_context(tc.tile_pool(name="psL", bufs=2, space="PSUM"))
        sbG = ctx.enter_context(tc.tile_pool(name="sbG", bufs=4))
        sbW = ctx.enter_context(tc.tile_pool(name="sbW", bufs=18))
        sbO = ctx.enter_context(tc.tile_pool(name="sbO", bufs=3))

        junk = const.tile([128, 512], BF16, name="junk", tag="junk")
        nc.vector.memset(junk[:], 0.0)

        es = const.tile([16, NCH * 128], BF16, name="es", tag="es")
        nc.gpsimd.dma_start(es[:], e_d[:])
        pts = const.tile([16, BC], BF16, name="pts", tag="pts")
        nc.gpsimd.dma_start(pts[:], pt_d[:])
        # xt on sync; weights/bias/S on scalar
        xta, xtb, wts = [], [], []
        for k in range(KF):
            t_ = const.tile([128, 1024], BF16, name=f"xta{k}", tag=f"xta{k}")
            nc.sync.dma_start(t_[:], xt_d[k * 128:(k + 1) * 128, 0:1024])
            xta.append(t_)
        for k in range(KF):
            t_ = const.tile([128, NCH * 128], BF16, name=f"wts{k}",
                            tag=f"wts{k}")
            nc.scalar.dma_start(t_[:], wt_d[k * 128:(k + 1) * 128, :])
            wts.append(t_)
        bts = const.tile([128, NCH], F32, name="bts", tag="bts")
        nc.scalar.dma_start(bts[:], bt_d[:])
        ss = []
        for i, sd in enumerate((s0_d, s1_d)):
            t_ = const.tile([128, C], BF16, name=f"ss{i}", tag=f"ss{i}")
            nc.scalar.dma_start(t_[:], sd[:])
            ss.append(t_)
        for k in range(KF):
            t_ = const.tile([128, 3072], BF16, name=f"xtb{k}", tag=f"xtb{k}")
            nc.sync.dma_start(t_[:], xt_d[k * 128:(k + 1) * 128, 1024:BC])
            xtb.append(t_)
        # HAM warmup: junk matmuls into a psG-pool tile (same tag -> no
        # extra PSUM footprint; the bank is recycled by the real GEMMs)
        warm = psG.tile([128, 1024], F32, name="psg", tag="psg")
        for _ in range(12):
            nc.tensor.matmul(warm[:, 0:512], junk[:, 0:128], junk[:],
                             start=True, stop=True)

        PHASES = [(0, 512), (512, 512), (1024, 1024), (2048, 1024),
                  (3072, 512), (3584, 512)]

        def x_slice(k, off, w):
            if off + w <= 1024:
                return xta[k][:, off:off + w]
            return xtb[k][:, off - 1024:off - 1024 + w]

        wtjs = {}
        obs = {}
        drain_ct = [0]

        def scatter_unit(ph, u):
            off, w_ph = PHASES[ph]
            bs, h = u // 2, u % 2
            sb = off // 128 + bs            # global 128-row subtile index
            grp = sb // 4                   # output DMA group of 4 subtiles
            if sb % 4 == 0 and h == 0:
                obs[grp] = sbO.tile([128, 4000], BF16, name="ob", tag="ob")
            ob = obs[grp]
            w = 512 if h == 0 else C - 512
            pl = psL.tile([128, 512], F32, name="pl", tag="pl")
            for (j, n0, n1, st, sp) in plans[h]:
                nc.tensor.matmul(pl[:, n0 - h * 512:n1 - h * 512],
                                 wtjs[(ph, j)][:, bs * 128:(bs + 1) * 128],
                                 ss[j % 2][:, n0:n1], start=st, stop=sp,
                                 skip_group_check=True)
            dst = ob[:, (sb % 4) * 1000 + h * 512:
                     (sb % 4) * 1000 + h * 512 + w]
            if drain_ct[0] % 2 == 0:
                nc.scalar.activation(dst, pl[:, 0:w],
                                     mybir.ActivationFunctionType.Identity,
                                     bias=0.0, scale=1.0)
            else:
                nc.vector.tensor_copy(dst, pl[:, 0:w])
            drain_ct[0] += 1
            if sb % 4 == 3 and h == 1:
                eng = nc.sync if grp % 2 == 0 else nc.scalar
                eng.dma_start(out_d[:, grp * 4000:(grp + 1) * 4000], ob[:])
                del obs[grp]

        for ph, (off, w_ph) in enumerate(PHASES):
            prev_units = ((w_ph and ph > 0) and
                          [(ph - 1, u) for u in range(PHASES[ph - 1][1] // 64)]
                          or [])
            per_j = (len(prev_units) + NCH - 1) // NCH if prev_units else 0
            ui = 0
            for j in range(NCH):
                jsl = bass.ts(j, 128)
                psg = psG.tile([128, 1024], F32, name="psg", tag="psg")
                for k in range(KF):
                    for h in range(w_ph // 512):
                        nc.tensor.matmul(
                            psg[:, h * 512:(h + 1) * 512],
                            wts[k][:, jsl], x_slice(k, off + h * 512, 512),
                            start=(k == 0), stop=(k == KF - 1))
                pm = psM.tile([128, 1024], F32, name="pm", tag="pm")
                for h in range(w_ph // 512):
                    nc.tensor.matmul(pm[:, h * 512:(h + 1) * 512],
                                     es[:, jsl],
                                     pts[:, off + h * 512:off + h * 512 + 512],
                                     start=True, stop=True)
                gb = sbG.tile([128, 1024], BF16, name="gb", tag="gb")
                nc.scalar.activation(gb[:, 0:w_ph], psg[:, 0:w_ph],
                                     mybir.ActivationFunctionType.Identity,
                                     bias=bts[:, j:j + 1], scale=1.0)
                wtj = sbW.tile([128, 1024], BF16, name="wtj", tag="wtj")
                nc.vector.tensor_mul(wtj[:, 0:w_ph], gb[:, 0:w_ph],
                                     pm[:, 0:w_ph])
                wtjs[(ph, j)] = wtj
                for _ in range(per_j):
                    if ui < len(prev_units):
                        scatter_unit(*prev_units[ui])
                        ui += 1
            while ui < len(prev_units):
                scatter_unit(*prev_units[ui])
                ui += 1
            if ph > 0:
                for jj in range(NCH):
                    del wtjs[(ph - 1, jj)]
        last = len(PHASES) - 1
        for u in range(PHASES[last][1] // 64):
            scatter_unit(last, u)
    nc.finalize()
    return nc


def kernel(features, group_probs, W, b, label_ids):
    global LAST_EXEC_NS
    features = np.asarray(features, dtype=np.float32)
    group_probs = np.asarray(group_probs, dtype=np.float32)
    prep = _host_prep(W, b, label_ids)
    nc = _build_program(prep["bands"])

    XT = np.ascontiguousarray(features.T.astype(ml_dtypes.bfloat16))
    PT = np.ascontiguousarray(group_probs.T.astype(ml_dtypes.bfloat16))
    in_maps = []
    for c in range(NCORE):
        in_maps.append({
            "xt": np.ascontiguousarray(XT[:, c * BC:(c + 1) * BC]),
            "pt": np.ascontiguousarray(PT[:, c * BC:(c + 1) * BC]),
            "wt": prep["WT"],
            "e": prep["E"],
            "bt": prep["biasT"],
            "s0": prep["S0"],
            "s1": prep["S1"],
        })

    trace = bool(os.environ.get("BASS_TRACE"))
    if trace:
        bass_utils.upload_artifacts = lambda d: "local://skipped"
    try:
        res = bass_utils.run_bass_kernel_spmd(nc, in_maps,
                                              core_ids=list(range(NCORE)))
    except Exception:
        # transient NRT device errors have been observed; one retry
        res = bass_utils.run_bass_kernel_spmd(nc, in_maps,
                                              core_ids=list(range(NCORE)))
    if trace:
        LAST_EXEC_NS = res.exec_time_ns
        if res.exec_time_ns is not None:
            print(f"HW exec time: {res.exec_time_ns} ns")
        if res.instructions_and_trace is not None:
            print(f"Trace path: {res.instructions_and_trace[1]}")
        if res.profile_json is not None:
            print(f"Profile json: {res.profile_json}")

    parts = []
    for c in range(NCORE):
        arr = np.asarray(res.results[c]["logits"]).astype(np.float32)
        arr = arr.reshape(128, NP, 2, 4, 1000)
        parts.append(arr.transpose(1, 2, 3, 0, 4).reshape(BC, C))
    return np.ascontiguousarray(np.concatenate(parts, axis=0))
